# revision 51
# baseline (speedup 1.0000x reference)
"""Trainium2 Bass kernel for nn_PairwiseAttentionTerminal — v2.

Cost-model-driven design (CoreSim charges: matmul = out-free-rows x cpr;
LDWEIGHTS free; ACT/DVE = free-size + access-penalty; Pool = free-size/eff,
SBUF only — the walrus verifier rejects GPSIMD<->PSUM; DMA charged to the
issuing queue):

  - The 64 exp(S) tiles [128,1024] are the wall: every S element crosses
    PSUM->SBUF exactly once through ACT (native Exp, ~1.04us/tile) or DVE
    (Schraudolph bit-trick exp, ~1.19us/tile).  Split ~54/46 by a Bresenham
    pattern with endpoint overrides (DVE-first while ACT does q/k copies,
    ACT-last to shorten the drain tail).
  - AV q-major with the DENOMINATOR MERGED into the same matmul: vaug holds
    33 columns per head (32 v + ones), one [128,33] MM per (qt,h,kk) into a
    bank-aligned psA [128,2,512].  Kills the separate denominator MMs.
  - gate = sigmoid(z) = 0.5*(1+tanh(z/2)): tanh rides the SAME act table as
    Exp ("exp_and_others"), projected q-major (no transposes); +1 on Pool
    (SBUF); the multiply into attn is a 2x-mode bf16 DVE tensor_tensor; the
    0.5 is folded into Wo host-side.
  - LN rstd via fp32 rsqrt bit-trick + 2 Newton steps on DVE (tiny [128,8]
    ops) — frees the act table from Ln so ONE table load serves the kernel,
    prefetched at t=0 behind the input DMA.
  - All q/k biases folded away (softmax shift-invariance) into the per-key
    bias projection; LN gamma/beta folded into every projection weight.
  - weights shipped bf16; everything bf16 on-chip except PSUM accumulators.

Sharding: batch B=8 -> one batch element per core, weights replicated.
"""

import numpy as np
from contextlib import ExitStack

L, B, F, H, C = 1024, 8, 256, 8, 32
HC = H * C
EPS = 1e-5
N_CORES = 8
P = 128
NLT = L // P   # 8 L-tiles
NFC = F // P   # 2 F-chunks
NHC = HC // P  # 2 hc-chunks
CA = C + 1     # v columns per head incl denominator ones column

A_SCH = float(128.0 / np.log(2.0))
B_SCH = float((127.0 - 0.057) * 128.0)  # rms-optimal shift
MAGIC = 0x5F3759DF

# exp tile engine split: True -> ACT native exp, False -> DVE Schraudolph.
ACT_EXP = 34  # of 64 tiles on ACT
# force the first tiles onto DVE (ACT busy with q/k copies).
FORCE_DVE_HEAD = 2
FORCE_ACT_TAIL = 1

_COMPILED = {}


def _exp_assignment():
    """64 bools: True = ACT.  Strict D,A alternation — engine-pattern
    mini-sim shows any doubled engine (AA/DD runs) serializes the 3-slot
    PSUM ring and costs ~15% stream throughput."""
    n = H * NLT
    return [(t % 2) == 1 for t in range(n)]


def _build():
    import concourse.bacc as bacc
    import concourse.mybir as mybir
    import concourse.tile as tile

    f32 = mybir.dt.float32
    bf16 = mybir.dt.bfloat16
    i16 = mybir.dt.int16
    i32 = mybir.dt.int32
    AF = mybir.ActivationFunctionType
    ALU = mybir.AluOpType

    nc = bacc.Bacc("TRN2", target_bir_lowering=False)

    feat_e = nc.dram_tensor("feat", [L, F], f32, kind="ExternalInput")
    wq_e = nc.dram_tensor("wq", [P, NFC, HC], bf16, kind="ExternalInput")
    wk_e = nc.dram_tensor("wk", [P, NFC, HC], bf16, kind="ExternalInput")
    wv_e = nc.dram_tensor("wv", [P, NFC, HC], bf16, kind="ExternalInput")
    wg_e = nc.dram_tensor("wg", [P, NFC, HC], bf16, kind="ExternalInput")
    wb_e = nc.dram_tensor("wb", [P, NFC, H], bf16, kind="ExternalInput")
    wo_e = nc.dram_tensor("wo", [P, NHC, F], bf16, kind="ExternalInput")
    bbb_e = nc.dram_tensor("bbb", [P, H], f32, kind="ExternalInput")
    bob_e = nc.dram_tensor("bob", [1, 2 * F], bf16, kind="ExternalInput")
    idb_e = nc.dram_tensor("idb", [P, P], bf16, kind="ExternalInput")
    out_e = nc.dram_tensor("out", [L, F], f32, kind="ExternalOutput")

    act_on = _exp_assignment()

    with tile.TileContext(nc) as tc, ExitStack() as ctx:
        const = ctx.enter_context(tc.tile_pool(name="const", bufs=1))
        main = ctx.enter_context(tc.tile_pool(name="main", bufs=1))
        work = ctx.enter_context(tc.tile_pool(name="work", bufs=4))
        epool = ctx.enter_context(tc.tile_pool(name="epool", bufs=12))
        opool = ctx.enter_context(tc.tile_pool(name="opool", bufs=6))

        # ---- input DMAs: feat split across SP/ACT/Pool queues so the SP
        # queue is free for the xT DMA-transposes; q/k weights ride the
        # otherwise-idle ACT queue.
        ftp = ctx.enter_context(tc.tile_pool(name="ftp", bufs=1))
        ftall = ftp.tile([P, NLT, F], f32, name="ftall")
        ft = [ftall[:, i, :] for i in range(NLT)]
        fview = feat_e.ap().rearrange("(i p) f -> p i f", p=P)
        nc.sync.dma_start(ftall[:, 0:2, :], fview[:, 0:2, :])
        nc.scalar.dma_start(ftall[:, 2:4, :], fview[:, 2:4, :])
        nc.gpsimd.dma_start(ftall[:, 4:6, :], fview[:, 4:6, :])
        nc.gpsimd.dma_start(ftall[:, 6:8, :], fview[:, 6:8, :])

        def load(name, ext, shape, dt_, eng=None):
            t = const.tile(shape, dt_, name=name)
            (eng or nc.sync).dma_start(t[:], ext.ap())
            return t

        wq = load("wq_s", wq_e, [P, NFC, HC], bf16, nc.scalar)
        wk = load("wk_s", wk_e, [P, NFC, HC], bf16, nc.scalar)
        identb = load("idb_s", idb_e, [P, P], bf16, nc.scalar)

        # ---- act-table prefetch: dummy exp behind the early ACT DMAs ----
        dummy = const.tile([1, 1], f32, name="dummy")
        nc.vector.memset(dummy[:], 0.0)
        dummy2 = const.tile([1, 1], bf16, name="dummy2")
        nc.scalar.activation(dummy2[:], dummy[:], AF.Exp)



        # ---- persistent SBUF ----
        xn = [main.tile([P, F], bf16, name=f"xn{i}") for i in range(NLT)]
        xT = [main.tile([P, L], bf16, name=f"xT{j}") for j in range(NFC)]
        qkT = [main.tile([P, 2, L], bf16, name=f"qkT{j}") for j in range(NFC)]
        qTs = [qkT[j][:, 0, :] for j in range(NFC)]
        kTs = [qkT[j][:, 1, :] for j in range(NFC)]
        gateall = main.tile([P, NLT, HC], bf16, name="gateall")
        vaug = main.tile([P, NLT, H * CA], bf16, name="vaug")
        # denominator ones columns (strided memset, 64 cols)
        nc.vector.memset(
            vaug[:].rearrange("p k (h c) -> p k h c", c=CA)[:, :, :, C], 1.0)
        bTsb = main.tile([P, NLT * H], f32, name="bTsb")
        sbT = main.tile([P, NLT * H], f32, name="sbT")
        agall = main.tile([P, NLT, HC], bf16, name="agall")
        agT = [main.tile([P, L], bf16, name=f"agT{j}") for j in range(NHC)]
        stat = main.tile([P, 48], f32, name="stat")

        psT_cm = tc.tile_pool(name="psT", bufs=2, space="PSUM")
        psT = psT_cm.__enter__()

        # ======= Stage A: LN (bn_stats on DVE, rstd Newton) =======
        # stat cols: [0:16] (mean,var) interleaved per tile; [16:24] rstd
        # bn_stats in 2-tile groups (free 512 == BN_STATS_FMAX), ONE Newton
        # chain for all 8 tiles, then per-tile xn + PE transpose with the
        # PSUM->SBUF copies split ACT (chunk 0) / DVE (chunk 1).
        for g in range(4):
            bns = work.tile([P, 2, 6], f32, tag="bns")
            nc.vector.bn_stats(bns[:], ftall[:, 2 * g:2 * g + 2, :])
            for t_ in range(2):
                i = 2 * g + t_
                nc.vector.bn_aggr(stat[:, 2 * i:2 * i + 2], bns[:, t_, :])
        var_ap = stat[:, 0:16].rearrange("p (i t) -> p i t", t=2)[:, :, 1]
        rs = stat[:, 16:24]
        ve = work.tile([P, 8], f32, tag="ve", name="ve")
        nc.vector.tensor_scalar(ve[:], var_ap, EPS, None, op0=ALU.add)
        # rsqrt bit-trick seed: y0 = MAGIC - (i >> 1)
        zi = work.tile([P, 8], i32, tag="zi", name="zi")
        nc.vector.tensor_scalar(zi[:], ve[:].bitcast(i32), 1, None,
                                op0=ALU.arith_shift_right)
        nc.vector.tensor_scalar(rs.bitcast(i32), zi[:], -1, MAGIC,
                                op0=ALU.mult, op1=ALU.add)
        # 2 Newton iterations: y = y*(1.5 - 0.5*x*y*y)
        uu = work.tile([P, 8], f32, tag="uu", name="uu")
        ww = work.tile([P, 8], f32, tag="ww", name="ww")
        for _ in range(2):
            nc.vector.tensor_tensor(uu[:], rs, rs, op=ALU.mult)
            nc.vector.scalar_tensor_tensor(ww[:], ve[:], -0.5, uu[:],
                                           op0=ALU.mult, op1=ALU.mult)
            nc.vector.scalar_tensor_tensor(rs, ww[:], 1.5, rs,
                                           op0=ALU.add, op1=ALU.mult)
        for i in range(NLT):
            # xn split across Pool (even) / DVE (odd) to halve the chain
            xeng = nc.gpsimd if i % 2 == 0 else nc.vector
            xeng.tensor_scalar(xn[i][:], ft[i], stat[:, 2 * i:2 * i + 1],
                              stat[:, 16 + i:17 + i],
                              op0=ALU.subtract, op1=ALU.mult)
            if i < 4:
                tp = psT.tile([P, 256], bf16, tag="t", name=f"tp{i}")
                for j in range(NFC):
                    nc.tensor.transpose(tp[:, j * P:(j + 1) * P],
                                        xn[i][:, j * P:(j + 1) * P],
                                        identb[:])
                    if j == 0:
                        nc.scalar.activation(xT[j][:, i * P:(i + 1) * P],
                                             tp[:, j * P:(j + 1) * P],
                                             AF.Copy)
                    else:
                        nc.vector.tensor_copy(xT[j][:, i * P:(i + 1) * P],
                                              tp[:, j * P:(j + 1) * P])
            else:
                # late tiles: DMA transpose, off-engine latency overlaps
                for j in range(NFC):
                    nc.sync.dma_start_transpose(
                        xT[j][:, i * P:(i + 1) * P],
                        xn[i][:, j * P:(j + 1) * P])

        # late weights on SP after the stage-A work is queued
        wb = load("wb_s", wb_e, [P, NFC, H], bf16)
        bbb = load("bbb_s", bbb_e, [P, H], f32)
        wv = load("wv_s", wv_e, [P, NFC, HC], bf16)
        wg = load("wg_s", wg_e, [P, NFC, HC], bf16)
        wo = load("wo_s", wo_e, [P, NHC, F], bf16)
        bob = load("bob_s", bob_e, [1, 2 * F], bf16)
        onesf = const.tile([1, P], bf16, name="onesf")
        nc.vector.memset(onesf[:], 1.0)

        psT_cm.__exit__(None, None, None)

        # ================= Stage B: projections =================
        # chunk-0 q/k (one fused [P,2,L] psum -> one ACT copy) and the
        # per-key bias are issued up front; chunk-1 q/k, v and the gate are
        # deferred into the attention stream (their PSUM comes from psS).
        psP_cm = tc.tile_pool(name="psP", bufs=4, space="PSUM")
        psP = psP_cm.__enter__()

        def proj_qk(j, pool, tag):
            # q and k chunk j, one INDEPENDENT [P,512] psum tile per
            # (qk, m) half so the proj matmuls never serialize against the
            # ACT copies (per-tile WAR tracking).  Order: k_m0, q_m0, q_m1,
            # k_m1 — the first QK needs k_m0 + q_m0 + q_m1.
            for qk, m in ((1, 0), (0, 0), (0, 1), (1, 1)):
                w_ = wq if qk == 0 else wk
                ps = pool.tile([P, 512], f32, tag=tag, name=f"pqk{j}_{qk}{m}")
                ms = slice(512 * m, 512 * (m + 1))
                for jj in range(NFC):
                    nc.tensor.matmul(ps[:],
                                     w_[:, jj, j * P:(j + 1) * P],
                                     xT[jj][:, ms],
                                     start=(jj == 0), stop=(jj == 1))
                if (qk, m) in ((0, 0), (1, 1)):
                    # q_m0 / k_m1 on DVE, k_m0 / q_m1 on ACT: two copies
                    # per engine run in parallel, halving the chain to the
                    # first QK (needs k_m0 + q_m0 + q_m1)
                    nc.vector.tensor_copy(qkT[j][:, qk, ms], ps[:])
                else:
                    nc.scalar.activation(qkT[j][:, qk, ms], ps[:], AF.Copy)

        def proj_qk_half(j, qk, pool, tag):
            # one of q/k, chunk j, via a [P, L] stream slot -> ACT copy
            w_ = wq if qk == 0 else wk
            ps = pool.tile([P, L], f32, tag=tag, name=f"pqk{j}_{qk}")
            for m in range(2):
                ms = slice(512 * m, 512 * (m + 1))
                for jj in range(NFC):
                    nc.tensor.matmul(ps[:, ms],
                                     w_[:, jj, j * P:(j + 1) * P],
                                     xT[jj][:, ms],
                                     start=(jj == 0), stop=(jj == 1))
            nc.scalar.activation(qkT[j][:, qk, :], ps[:], AF.Copy)

        def proj_gate(pr, pool, tag):
            # gate q-major like v: [P, 2, HC] per 2-tile group; tanh(z/2)
            ps = pool.tile([P, 2, HC], f32, tag=tag, name=f"pg{pr}")
            for t_ in range(2):
                i = 2 * pr + t_
                for jj in range(NFC):
                    nc.tensor.matmul(ps[:, t_, :],
                                     xT[jj][:, i * P:(i + 1) * P],
                                     wg[:, jj, :],
                                     start=(jj == 0), stop=(jj == 1))
            dst = gateall[:, 2 * pr:2 * pr + 2, :]
            nc.scalar.activation(dst, ps[:], AF.Tanh, scale=0.5)
            # +1 on Pool (SBUF only): gate := tanh+1  in [0,2]
            nc.gpsimd.tensor_scalar(dst, dst, 1.0, None, op0=ALU.add)

        def proj_v(pr, pool, tag):
            ps = pool.tile([P, 2, HC], f32, tag=tag, name=f"pv{pr}")
            for t_ in range(2):
                i = 2 * pr + t_
                for jj in range(NFC):
                    nc.tensor.matmul(ps[:, t_, :],
                                     xT[jj][:, i * P:(i + 1) * P],
                                     wv[:, jj, :],
                                     start=(jj == 0), stop=(jj == 1))
            dst = (vaug[:, 2 * pr:2 * pr + 2, :]
                   .rearrange("p t (h c) -> p t h c", c=CA)[:, :, :, 0:C])
            src = ps[:].rearrange("p t (h c) -> p t h c", c=C)
            nc.scalar.activation(dst, src, AF.Copy)

        # per-key bias projection FIRST (all 8 L-tiles into one PSUM bank):
        # the exp bias chain (psB -> bTsb -> sbT) must be ready before the
        # first exp, so it precedes the q/k projections on the PE queue.
        psB = psP.tile([P, 64], f32, tag="pb", name="pb")
        for i in range(NLT):
            for jj in range(NFC):
                nc.tensor.matmul(psB[:, i * H:(i + 1) * H],
                                 xT[jj][:, i * P:(i + 1) * P],
                                 wb[:, jj, :], start=(jj == 0), stop=(jj == 1))
        nc.vector.tensor_tensor(
            bTsb[:].rearrange("p (k h) -> p k h", k=NLT),
            psB[:].rearrange("p (k h) -> p k h", k=NLT),
            bbb[:].unsqueeze(1).broadcast_to([P, NLT, H]), op=ALU.add)
        nc.gpsimd.tensor_scalar(sbT[:], bTsb[:], A_SCH, B_SCH,
                                op0=ALU.mult, op1=ALU.add)

        proj_qk(0, psP, "p")

        psP_cm.__exit__(None, None, None)

        # ================= Stage C: attention =================
        # psS = 3 x [P,1024] (6 banks) QK->exp ring; psA = 1 bank, one HEAD
        # at a time (denominator rides column 32 of each 33-col AV group);
        # psD = 1 bank for the deferred v/gate/qk1 projections so they never
        # steal a psS slot.  Deferred work is chunked <=2 matmuls per stream
        # position so the in-order PE queue never delays a QK by more than
        # ~0.4us.  Drains split: normalize right after the head's last AV;
        # the pool gate-multiply waits until all gate groups exist.
        psA_cm = tc.tile_pool(name="psA", bufs=1, space="PSUM")
        psA = psA_cm.__enter__()
        psD_cm = tc.tile_pool(name="psD", bufs=1, space="PSUM")
        psD = psD_cm.__enter__()
        psS_cm = tc.tile_pool(name="psS", bufs=3, space="PSUM")
        psS = psS_cm.__enter__()

        psA_t = {}
        eT = {}

        def issue_av(h, kk):
            if h not in psA_t:
                psA_t[h] = psA.tile([P, 512], f32, tag="a", name=f"pa{h}")
            pa = psA_t[h]
            e = eT[(h, kk)]
            first = (kk == 0)
            last = (kk == NLT - 1)
            for qt in range(NLT):
                lhs = e[:, qt * P:(qt + 1) * P]
                nc.tensor.matmul(pa[:, qt * CA:(qt + 1) * CA], lhs,
                                 vaug[:, kk, h * CA:(h + 1) * CA],
                                 start=(first and qt == 0),
                                 stop=(last and qt == NLT - 1))

        def norm_head(h):
            """attn = pa / denom -> agall (ungated)."""
            pa = psA_t[h]
            pav = pa[:, 0:NLT * CA].rearrange("p (q c) -> p q c", c=CA)
            rec = work.tile([P, NLT], f32, tag="rec", name=f"rec{h}")
            nc.vector.reciprocal(rec[:], pav[:, :, C])
            cs = slice(h * C, (h + 1) * C)
            nc.vector.tensor_tensor(
                agall[:, :, cs], pav[:, :, 0:C],
                rec[:].unsqueeze(2).broadcast_to([P, NLT, C]), op=ALU.mult)

        def gate_head(h, eng):
            cs = slice(h * C, (h + 1) * C)
            eng.tensor_tensor(agall[:, :, cs], agall[:, :, cs],
                              gateall[:, :, cs], op=ALU.mult)

        def fine_tail(h):
            """head 7: per-qt normalize+gate+transpose+out."""
            pa = psA_t[h]
            pav = pa[:, 0:NLT * CA].rearrange("p (q c) -> p q c", c=CA)
            rec = work.tile([P, NLT], f32, tag="rec", name=f"rec{h}")
            nc.vector.reciprocal(rec[:], pav[:, :, C])
            cs = slice(h * C, (h + 1) * C)
            for qt in range(NLT):
                agv = agall[:, qt:qt + 1, cs]
                nc.vector.tensor_tensor(
                    agv, pav[:, qt:qt + 1, 0:C],
                    rec[:, qt:qt + 1].unsqueeze(2).broadcast_to([P, 1, C]),
                    op=ALU.mult)
                nc.vector.tensor_tensor(agv, agv, gateall[:, qt:qt + 1, cs],
                                        op=ALU.mult)
                tp = psS.tile([P, P], bf16, tag="s", name=f"tp{qt}")
                nc.tensor.transpose(tp[:], agall[:, qt, P:2 * P], identb[:])
                # tp copy on ACT: DVE stays on the norm/gate chain
                nc.scalar.activation(agT[1][:, qt * P:(qt + 1) * P], tp[:],
                                     AF.Copy)
                out_tile(qt)

        def out_tile(i):
            ps = psS.tile([P, 256], f32, tag="s", name=f"po{i}")
            nc.tensor.matmul(ps[:], onesf[:], bob[:, 0:256],
                             start=True, stop=False)
            for j in range(NHC):
                nc.tensor.matmul(ps[:], agT[j][:, i * P:(i + 1) * P],
                                 wo[:, j, :], start=False, stop=(j == 1))
            o = opool.tile([P, 256], f32, tag="o", name=f"ot{i}")
            # alternate the PSUM->SBUF copy between ACT and DVE in the tail
            if i % 2 == 0:
                nc.scalar.activation(o[:], ps[:], AF.Copy)
            else:
                nc.vector.tensor_copy(o[:], ps[:])
            eng = nc.gpsimd if i % 2 == 0 else nc.sync
            eng.dma_start(out_e.ap()[i * P:(i + 1) * P, :], o[:])

        # ---- deferred projection chunks (psD, <=2 MMs each) ----
        psD_t = {}

        def v_chunk(pr, half):
            # half 0: tile 2pr MMs; half 1: tile 2pr+1 MMs + strided copy
            if half == 0:
                psD_t["v"] = psD.tile([P, 2, HC], f32, tag="d", name=f"pv{pr}")
            ps = psD_t["v"]
            i = 2 * pr + half
            for jj in range(NFC):
                nc.tensor.matmul(ps[:, half, :],
                                 xT[jj][:, i * P:(i + 1) * P], wv[:, jj, :],
                                 start=(jj == 0), stop=(jj == 1))
            if half == 1:
                dst = (vaug[:, 2 * pr:2 * pr + 2, :]
                       .rearrange("p t (h c) -> p t h c", c=CA)[:, :, :, 0:C])
                src = ps[:].rearrange("p t (h c) -> p t h c", c=C)
                nc.scalar.activation(dst, src, AF.Copy)

        def g_chunk(pr, half):
            if half == 0:
                psD_t["g"] = psD.tile([P, 2, HC], f32, tag="d", name=f"pg{pr}")
            ps = psD_t["g"]
            i = 2 * pr + half
            for jj in range(NFC):
                nc.tensor.matmul(ps[:, half, :],
                                 xT[jj][:, i * P:(i + 1) * P], wg[:, jj, :],
                                 start=(jj == 0), stop=(jj == 1))
            if half == 1:
                dst = gateall[:, 2 * pr:2 * pr + 2, :]
                nc.scalar.activation(dst, ps[:], AF.Tanh, scale=0.5)
                nc.gpsimd.tensor_scalar(dst, dst, 1.0, None, op0=ALU.add)

        def qk1_chunk(qk, m):
            # one m-half of q/k chunk 1 (2 MMs) + its copy
            w_ = wq if qk == 0 else wk
            ps = psD.tile([P, 512], f32, tag="d", name=f"pqk1_{qk}{m}")
            ms = slice(512 * m, 512 * (m + 1))
            for jj in range(NFC):
                nc.tensor.matmul(ps[:], w_[:, jj, P:2 * P], xT[jj][:, ms],
                                 start=(jj == 0), stop=(jj == 1))
            nc.scalar.activation(qkT[1][:, qk, ms], ps[:], AF.Copy)

        extras = {
            (0, 0): lambda: v_chunk(0, 0), (0, 1): lambda: v_chunk(0, 1),
            (0, 2): lambda: v_chunk(1, 0), (0, 3): lambda: v_chunk(1, 1),
            (0, 4): lambda: v_chunk(2, 0), (0, 5): lambda: v_chunk(2, 1),
            (0, 6): lambda: v_chunk(3, 0), (0, 7): lambda: v_chunk(3, 1),
            (1, 0): lambda: g_chunk(0, 0), (1, 1): lambda: g_chunk(0, 1),
            (1, 2): lambda: g_chunk(1, 0), (1, 3): lambda: g_chunk(1, 1),
            (1, 4): lambda: g_chunk(2, 0), (1, 5): lambda: g_chunk(2, 1),
            (1, 6): lambda: g_chunk(3, 0), (1, 7): lambda: g_chunk(3, 1),
            (2, 5): lambda: qk1_chunk(1, 0), (2, 7): lambda: qk1_chunk(1, 1),
            (3, 1): lambda: qk1_chunk(0, 0), (3, 5): lambda: qk1_chunk(0, 1),
            # gate multiplies (pool) after norm_head(h) (pops at (h+1,2))
            (2, 1): lambda: gate_head(0, nc.gpsimd),
            (2, 3): lambda: gate_head(1, nc.gpsimd),
            (3, 3): lambda: gate_head(2, nc.gpsimd),
            (4, 3): lambda: gate_head(3, nc.gpsimd),
            (5, 3): lambda: gate_head(4, nc.gpsimd),
            (6, 3): lambda: gate_head(5, nc.gpsimd),
            (7, 3): lambda: gate_head(6, nc.gpsimd),
        }
        # agT[0] DMA transposes once heads 0..3 are gated
        def agt0(q0, q1):
            for qt in range(q0, q1):
                nc.sync.dma_start_transpose(agT[0][:, qt * P:(qt + 1) * P],
                                            agall[:, qt, 0:P])
        extras[(5, 2)] = lambda: agt0(0, 4)
        extras[(5, 4)] = lambda: agt0(4, 8)

        # AV issued with LAG 2 behind the exp stream: the PE queue is
        # in-order, so an AV waiting on exp(i-1) would block QK(i+1) whose
        # own dependency (slot of exp(i-2)) is already satisfied.
        pending = []

        def tick_av():
            t = pending.pop(0)
            issue_av(*t)
            if t[1] == NLT - 1 and t[0] < H - 1:
                norm_head(t[0])

        for h in range(H):
            jh, ph = h // 4, 32 * (h % 4)
            hp = slice(ph, ph + 32)
            for kk in range(NLT):
                sp = psS.tile([P, L], f32, tag="s", name=f"sp{h}_{kk}")
                for m in range(2):
                    ms = slice(512 * m, 512 * (m + 1))
                    nc.tensor.matmul(sp[:, ms],
                                     kTs[jh][hp, kk * P:(kk + 1) * P],
                                     qTs[jh][hp, ms], start=True, stop=True,
                                     tile_position=(ph, 0))
                e = epool.tile([P, L], bf16, tag="e", name=f"e{h}_{kk}")
                bcol = kk * H + h
                if act_on[h * NLT + kk]:
                    nc.scalar.activation(e[:], sp[:], AF.Exp,
                                         bias=bTsb[:, bcol:bcol + 1])
                else:
                    nc.vector.tensor_scalar(e[:].bitcast(i16), sp[:], A_SCH,
                                            sbT[:, bcol:bcol + 1],
                                            op0=ALU.mult, op1=ALU.add)
                eT[(h, kk)] = e
                pending.append((h, kk))
                if len(pending) > 3:
                    tick_av()
                if (h, kk) in extras:
                    extras[(h, kk)]()
        while pending:
            tick_av()
        fine_tail(H - 1)

        psS_cm.__exit__(None, None, None)
        psD_cm.__exit__(None, None, None)
        psA_cm.__exit__(None, None, None)

    # Pin Exp/Tanh/Copy to the one combined table set (single load).
    import concourse.bacc as bacc_mod
    orig_gat = bacc_mod.get_activation_tables

    def gat_combined(arch):
        t = orig_gat(arch)
        return {name: (funcs if name == "exp_and_others" else set())
                for name, funcs in t.items()}

    bacc_mod.get_activation_tables = gat_combined
    try:
        nc.compile()
    finally:
        bacc_mod.get_activation_tables = orig_gat
    return nc


def _prep_inputs(features, ln_g, ln_b, Wq, bq, Wk, bk, Wv, bv, Wb, bb,
                 Wg, bg, Wo, bo):
    import ml_dtypes
    bf = ml_dtypes.bfloat16
    f32 = np.float32
    sq = f32(1.0 / np.sqrt(C))
    g_ = np.asarray(ln_g, f32)[:, None]
    beta = np.asarray(ln_b, f32)

    Wq_ = np.asarray(Wq, f32) * g_ * sq
    Wk_ = np.asarray(Wk, f32) * g_
    Wv_ = np.asarray(Wv, f32) * g_
    Wg_ = np.asarray(Wg, f32) * g_
    bq_t = (beta @ np.asarray(Wq, f32) + np.asarray(bq, f32)) * sq  # [HC]
    bv_ = beta @ np.asarray(Wv, f32) + np.asarray(bv, f32)
    bg_ = beta @ np.asarray(Wg, f32) + np.asarray(bg, f32)
    assert np.abs(bv_).max() == 0.0, "nonzero v bias path not built"
    assert np.abs(bg_).max() == 0.0, "nonzero gate bias path not built"
    # per-key bias: Wb fold + q-bias cross term (softmax-invariant parts drop)
    WB = np.asarray(Wb, f32) * g_
    for h in range(H):
        WB[:, h] += Wk_[:, C * h:C * (h + 1)] @ bq_t[C * h:C * (h + 1)]
    BB = beta @ np.asarray(Wb, f32) + np.asarray(bb, f32)  # [H]

    def wsplit(W, n, dt_):
        return np.ascontiguousarray(
            np.asarray(W, f32).reshape(NFC, P, n).transpose(1, 0, 2)).astype(dt_)

    common = {
        "wq": wsplit(Wq_, HC, bf),
        "wk": wsplit(Wk_, HC, bf),
        "wv": wsplit(Wv_, HC, bf),
        "wg": wsplit(Wg_, HC, bf),
        "wb": wsplit(WB, H, bf),
        "wo": wsplit(np.asarray(Wo, f32) * 0.5, F, bf),
        "bbb": np.ascontiguousarray(np.tile(BB, (P, 1))).astype(f32),
        "idb": np.eye(P, dtype=np.float32).astype(bf),
        "bob": np.ascontiguousarray(
            np.tile(np.asarray(bo, f32), (1, 2))).astype(bf),
    }
    feats = np.asarray(features, f32)
    in_maps = []
    for b_ in range(N_CORES):
        m = dict(common)
        m["feat"] = np.ascontiguousarray(feats[:, b_, :])
        in_maps.append(m)
    return in_maps


def kernel(**inputs):
    from concourse.bass_utils import run_bass_kernel_spmd

    if "nc" not in _COMPILED:
        _COMPILED["nc"] = _build()
    nc = _COMPILED["nc"]
    in_maps = _prep_inputs(**inputs)
    res = run_bass_kernel_spmd(nc, in_maps, list(range(N_CORES)))
    out = np.stack([res.results[b_]["out"] for b_ in range(N_CORES)], axis=1)
    return np.ascontiguousarray(out.astype(np.float32))


# revision 53
# speedup vs baseline: 1.0042x; 1.0042x over previous
"""Trainium2 Bass kernel for nn_PairwiseAttentionTerminal — v2.

Cost-model-driven design (CoreSim charges: matmul = out-free-rows x cpr;
LDWEIGHTS free; ACT/DVE = free-size + access-penalty; Pool = free-size/eff,
SBUF only — the walrus verifier rejects GPSIMD<->PSUM; DMA charged to the
issuing queue):

  - The 64 exp(S) tiles [128,1024] are the wall: every S element crosses
    PSUM->SBUF exactly once through ACT (native Exp, ~1.04us/tile) or DVE
    (Schraudolph bit-trick exp, ~1.19us/tile).  Split ~54/46 by a Bresenham
    pattern with endpoint overrides (DVE-first while ACT does q/k copies,
    ACT-last to shorten the drain tail).
  - AV q-major with the DENOMINATOR MERGED into the same matmul: vaug holds
    33 columns per head (32 v + ones), one [128,33] MM per (qt,h,kk) into a
    bank-aligned psA [128,2,512].  Kills the separate denominator MMs.
  - gate = sigmoid(z) = 0.5*(1+tanh(z/2)): tanh rides the SAME act table as
    Exp ("exp_and_others"), projected q-major (no transposes); +1 on Pool
    (SBUF); the multiply into attn is a 2x-mode bf16 DVE tensor_tensor; the
    0.5 is folded into Wo host-side.
  - LN rstd via fp32 rsqrt bit-trick + 2 Newton steps on DVE (tiny [128,8]
    ops) — frees the act table from Ln so ONE table load serves the kernel,
    prefetched at t=0 behind the input DMA.
  - All q/k biases folded away (softmax shift-invariance) into the per-key
    bias projection; LN gamma/beta folded into every projection weight.
  - weights shipped bf16; everything bf16 on-chip except PSUM accumulators.

Sharding: batch B=8 -> one batch element per core, weights replicated.
"""

import numpy as np
from contextlib import ExitStack

L, B, F, H, C = 1024, 8, 256, 8, 32
HC = H * C
EPS = 1e-5
N_CORES = 8
P = 128
NLT = L // P   # 8 L-tiles
NFC = F // P   # 2 F-chunks
NHC = HC // P  # 2 hc-chunks
CA = C + 1     # v columns per head incl denominator ones column

A_SCH = float(128.0 / np.log(2.0))
B_SCH = float((127.0 - 0.057) * 128.0)  # rms-optimal shift
MAGIC = 0x5F3759DF

# exp tile engine split: True -> ACT native exp, False -> DVE Schraudolph.
ACT_EXP = 34  # of 64 tiles on ACT
# force the first tiles onto DVE (ACT busy with q/k copies).
FORCE_DVE_HEAD = 2
FORCE_ACT_TAIL = 1

_COMPILED = {}


def _exp_assignment():
    """64 bools: True = ACT.  Strict D,A alternation — engine-pattern
    mini-sim shows any doubled engine (AA/DD runs) serializes the 3-slot
    PSUM ring and costs ~15% stream throughput."""
    n = H * NLT
    return [(t % 2) == 1 for t in range(n)]


def _build():
    import concourse.bacc as bacc
    import concourse.mybir as mybir
    import concourse.tile as tile

    f32 = mybir.dt.float32
    bf16 = mybir.dt.bfloat16
    i16 = mybir.dt.int16
    i32 = mybir.dt.int32
    AF = mybir.ActivationFunctionType
    ALU = mybir.AluOpType

    nc = bacc.Bacc("TRN2", target_bir_lowering=False)

    feat_e = nc.dram_tensor("feat", [L, F], f32, kind="ExternalInput")
    wq_e = nc.dram_tensor("wq", [P, NFC, HC], bf16, kind="ExternalInput")
    wk_e = nc.dram_tensor("wk", [P, NFC, HC], bf16, kind="ExternalInput")
    wv_e = nc.dram_tensor("wv", [P, NFC, HC], bf16, kind="ExternalInput")
    wg_e = nc.dram_tensor("wg", [P, NFC, HC], bf16, kind="ExternalInput")
    wb_e = nc.dram_tensor("wb", [P, NFC, H], bf16, kind="ExternalInput")
    wo_e = nc.dram_tensor("wo", [P, NHC, F], bf16, kind="ExternalInput")
    bbb_e = nc.dram_tensor("bbb", [P, H], f32, kind="ExternalInput")
    bob_e = nc.dram_tensor("bob", [1, 2 * F], bf16, kind="ExternalInput")
    idb_e = nc.dram_tensor("idb", [P, P], bf16, kind="ExternalInput")
    out_e = nc.dram_tensor("out", [L, F], f32, kind="ExternalOutput")

    act_on = _exp_assignment()

    with tile.TileContext(nc) as tc, ExitStack() as ctx:
        const = ctx.enter_context(tc.tile_pool(name="const", bufs=1))
        main = ctx.enter_context(tc.tile_pool(name="main", bufs=1))
        work = ctx.enter_context(tc.tile_pool(name="work", bufs=4))
        epool = ctx.enter_context(tc.tile_pool(name="epool", bufs=12))
        opool = ctx.enter_context(tc.tile_pool(name="opool", bufs=6))

        # ---- input DMAs: feat split across SP/ACT/Pool queues so the SP
        # queue is free for the xT DMA-transposes; q/k weights ride the
        # otherwise-idle ACT queue.
        ftp = ctx.enter_context(tc.tile_pool(name="ftp", bufs=1))
        ftall = ftp.tile([P, NLT, F], f32, name="ftall")
        ft = [ftall[:, i, :] for i in range(NLT)]
        fview = feat_e.ap().rearrange("(i p) f -> p i f", p=P)
        nc.sync.dma_start(ftall[:, 0:2, :], fview[:, 0:2, :])
        nc.scalar.dma_start(ftall[:, 2:4, :], fview[:, 2:4, :])
        nc.gpsimd.dma_start(ftall[:, 4:6, :], fview[:, 4:6, :])
        nc.gpsimd.dma_start(ftall[:, 6:8, :], fview[:, 6:8, :])

        def load(name, ext, shape, dt_, eng=None):
            t = const.tile(shape, dt_, name=name)
            (eng or nc.sync).dma_start(t[:], ext.ap())
            return t

        wq = load("wq_s", wq_e, [P, NFC, HC], bf16, nc.scalar)
        wk = load("wk_s", wk_e, [P, NFC, HC], bf16, nc.scalar)
        identb = load("idb_s", idb_e, [P, P], bf16, nc.scalar)

        # ---- act-table prefetch: dummy exp behind the early ACT DMAs ----
        dummy = const.tile([1, 1], f32, name="dummy")
        nc.vector.memset(dummy[:], 0.0)
        dummy2 = const.tile([1, 1], bf16, name="dummy2")
        nc.scalar.activation(dummy2[:], dummy[:], AF.Exp)



        # ---- persistent SBUF ----
        xn = [main.tile([P, F], bf16, name=f"xn{i}") for i in range(NLT)]
        xT = [main.tile([P, L], bf16, name=f"xT{j}") for j in range(NFC)]
        qkT = [main.tile([P, 2, L], bf16, name=f"qkT{j}") for j in range(NFC)]
        qTs = [qkT[j][:, 0, :] for j in range(NFC)]
        kTs = [qkT[j][:, 1, :] for j in range(NFC)]
        gateall = main.tile([P, NLT, HC], bf16, name="gateall")
        vaug = main.tile([P, NLT, H * CA], bf16, name="vaug")
        # denominator ones columns (strided memset, 64 cols)
        nc.vector.memset(
            vaug[:].rearrange("p k (h c) -> p k h c", c=CA)[:, :, :, C], 1.0)
        bTsb = main.tile([P, NLT * H], f32, name="bTsb")
        sbT = main.tile([P, NLT * H], f32, name="sbT")
        agall = main.tile([P, NLT, HC], bf16, name="agall")
        agT = [main.tile([P, L], bf16, name=f"agT{j}") for j in range(NHC)]
        stat = main.tile([P, 48], f32, name="stat")

        psT_cm = tc.tile_pool(name="psT", bufs=2, space="PSUM")
        psT = psT_cm.__enter__()

        # ======= Stage A: LN (bn_stats on DVE, rstd Newton) =======
        # stat cols: [0:16] (mean,var) interleaved per tile; [16:24] rstd
        # bn_stats in 2-tile groups (free 512 == BN_STATS_FMAX), ONE Newton
        # chain for all 8 tiles, then per-tile xn + PE transpose with the
        # PSUM->SBUF copies split ACT (chunk 0) / DVE (chunk 1).
        bns = work.tile([P, 8, 6], f32, tag="bns")
        for i in range(NLT):
            nc.vector.bn_stats(bns[:, i, :], ft[i])
            nc.vector.bn_aggr(stat[:, 2 * i:2 * i + 2], bns[:, i, :])
        var_ap = stat[:, 0:16].rearrange("p (i t) -> p i t", t=2)[:, :, 1]
        rs = stat[:, 16:24]
        ve = work.tile([P, 8], f32, tag="ve", name="ve")
        nc.vector.tensor_scalar(ve[:], var_ap, EPS, None, op0=ALU.add)
        # rsqrt bit-trick seed: y0 = MAGIC - (i >> 1)
        zi = work.tile([P, 8], i32, tag="zi", name="zi")
        nc.vector.tensor_scalar(zi[:], ve[:].bitcast(i32), 1, None,
                                op0=ALU.arith_shift_right)
        nc.vector.tensor_scalar(rs.bitcast(i32), zi[:], -1, MAGIC,
                                op0=ALU.mult, op1=ALU.add)
        # 2 Newton iterations: y = y*(1.5 - 0.5*x*y*y)
        uu = work.tile([P, 8], f32, tag="uu", name="uu")
        ww = work.tile([P, 8], f32, tag="ww", name="ww")
        for _ in range(2):
            nc.vector.tensor_tensor(uu[:], rs, rs, op=ALU.mult)
            nc.vector.scalar_tensor_tensor(ww[:], ve[:], -0.5, uu[:],
                                           op0=ALU.mult, op1=ALU.mult)
            nc.vector.scalar_tensor_tensor(rs, ww[:], 1.5, rs,
                                           op0=ALU.add, op1=ALU.mult)
        for i in range(NLT):
            # xn split across Pool (even) / DVE (odd) to halve the chain
            xeng = nc.gpsimd if i % 2 == 0 else nc.vector
            xeng.tensor_scalar(xn[i][:], ft[i], stat[:, 2 * i:2 * i + 1],
                              stat[:, 16 + i:17 + i],
                              op0=ALU.subtract, op1=ALU.mult)
            if i < 4:
                tp = psT.tile([P, 256], bf16, tag="t", name=f"tp{i}")
                for j in range(NFC):
                    nc.tensor.transpose(tp[:, j * P:(j + 1) * P],
                                        xn[i][:, j * P:(j + 1) * P],
                                        identb[:])
                    if j == 0:
                        nc.scalar.activation(xT[j][:, i * P:(i + 1) * P],
                                             tp[:, j * P:(j + 1) * P],
                                             AF.Copy)
                    else:
                        nc.vector.tensor_copy(xT[j][:, i * P:(i + 1) * P],
                                              tp[:, j * P:(j + 1) * P])
            else:
                # late tiles: DMA transpose, off-engine latency overlaps
                for j in range(NFC):
                    nc.sync.dma_start_transpose(
                        xT[j][:, i * P:(i + 1) * P],
                        xn[i][:, j * P:(j + 1) * P])

        # late weights on SP after the stage-A work is queued
        wb = load("wb_s", wb_e, [P, NFC, H], bf16)
        bbb = load("bbb_s", bbb_e, [P, H], f32)
        wv = load("wv_s", wv_e, [P, NFC, HC], bf16)
        wg = load("wg_s", wg_e, [P, NFC, HC], bf16)
        wo = load("wo_s", wo_e, [P, NHC, F], bf16)
        bob = load("bob_s", bob_e, [1, 2 * F], bf16)
        onesf = const.tile([1, P], bf16, name="onesf")
        nc.vector.memset(onesf[:], 1.0)

        psT_cm.__exit__(None, None, None)

        # ================= Stage B: projections =================
        # chunk-0 q/k (one fused [P,2,L] psum -> one ACT copy) and the
        # per-key bias are issued up front; chunk-1 q/k, v and the gate are
        # deferred into the attention stream (their PSUM comes from psS).
        psP_cm = tc.tile_pool(name="psP", bufs=4, space="PSUM")
        psP = psP_cm.__enter__()

        def proj_qk(j, pool, tag):
            # q and k chunk j, one INDEPENDENT [P,512] psum tile per
            # (qk, m) half so the proj matmuls never serialize against the
            # ACT copies (per-tile WAR tracking).  Order: k_m0, q_m0, q_m1,
            # k_m1 — the first QK needs k_m0 + q_m0 + q_m1.
            for qk, m in ((1, 0), (0, 0), (0, 1), (1, 1)):
                w_ = wq if qk == 0 else wk
                ps = pool.tile([P, 512], f32, tag=tag, name=f"pqk{j}_{qk}{m}")
                ms = slice(512 * m, 512 * (m + 1))
                for jj in range(NFC):
                    nc.tensor.matmul(ps[:],
                                     w_[:, jj, j * P:(j + 1) * P],
                                     xT[jj][:, ms],
                                     start=(jj == 0), stop=(jj == 1))
                if (qk, m) in ((0, 0), (1, 1)):
                    # q_m0 / k_m1 on DVE, k_m0 / q_m1 on ACT: two copies
                    # per engine run in parallel, halving the chain to the
                    # first QK (needs k_m0 + q_m0 + q_m1)
                    nc.vector.tensor_copy(qkT[j][:, qk, ms], ps[:])
                else:
                    nc.scalar.activation(qkT[j][:, qk, ms], ps[:], AF.Copy)

        def proj_qk_half(j, qk, pool, tag):
            # one of q/k, chunk j, via a [P, L] stream slot -> ACT copy
            w_ = wq if qk == 0 else wk
            ps = pool.tile([P, L], f32, tag=tag, name=f"pqk{j}_{qk}")
            for m in range(2):
                ms = slice(512 * m, 512 * (m + 1))
                for jj in range(NFC):
                    nc.tensor.matmul(ps[:, ms],
                                     w_[:, jj, j * P:(j + 1) * P],
                                     xT[jj][:, ms],
                                     start=(jj == 0), stop=(jj == 1))
            nc.scalar.activation(qkT[j][:, qk, :], ps[:], AF.Copy)

        def proj_gate(pr, pool, tag):
            # gate q-major like v: [P, 2, HC] per 2-tile group; tanh(z/2)
            ps = pool.tile([P, 2, HC], f32, tag=tag, name=f"pg{pr}")
            for t_ in range(2):
                i = 2 * pr + t_
                for jj in range(NFC):
                    nc.tensor.matmul(ps[:, t_, :],
                                     xT[jj][:, i * P:(i + 1) * P],
                                     wg[:, jj, :],
                                     start=(jj == 0), stop=(jj == 1))
            dst = gateall[:, 2 * pr:2 * pr + 2, :]
            nc.scalar.activation(dst, ps[:], AF.Tanh, scale=0.5)
            # +1 on Pool (SBUF only): gate := tanh+1  in [0,2]
            nc.gpsimd.tensor_scalar(dst, dst, 1.0, None, op0=ALU.add)

        def proj_v(pr, pool, tag):
            ps = pool.tile([P, 2, HC], f32, tag=tag, name=f"pv{pr}")
            for t_ in range(2):
                i = 2 * pr + t_
                for jj in range(NFC):
                    nc.tensor.matmul(ps[:, t_, :],
                                     xT[jj][:, i * P:(i + 1) * P],
                                     wv[:, jj, :],
                                     start=(jj == 0), stop=(jj == 1))
            dst = (vaug[:, 2 * pr:2 * pr + 2, :]
                   .rearrange("p t (h c) -> p t h c", c=CA)[:, :, :, 0:C])
            src = ps[:].rearrange("p t (h c) -> p t h c", c=C)
            nc.scalar.activation(dst, src, AF.Copy)

        # per-key bias projection FIRST (all 8 L-tiles into one PSUM bank):
        # the exp bias chain (psB -> bTsb -> sbT) must be ready before the
        # first exp, so it precedes the q/k projections on the PE queue.
        psB = psP.tile([P, 64], f32, tag="pb", name="pb")
        for i in range(NLT):
            for jj in range(NFC):
                nc.tensor.matmul(psB[:, i * H:(i + 1) * H],
                                 xT[jj][:, i * P:(i + 1) * P],
                                 wb[:, jj, :], start=(jj == 0), stop=(jj == 1))
        nc.vector.tensor_tensor(
            bTsb[:].rearrange("p (k h) -> p k h", k=NLT),
            psB[:].rearrange("p (k h) -> p k h", k=NLT),
            bbb[:].unsqueeze(1).broadcast_to([P, NLT, H]), op=ALU.add)
        nc.gpsimd.tensor_scalar(sbT[:], bTsb[:], A_SCH, B_SCH,
                                op0=ALU.mult, op1=ALU.add)

        proj_qk(0, psP, "p")

        psP_cm.__exit__(None, None, None)

        # ================= Stage C: attention =================
        # psS = 3 x [P,1024] (6 banks) QK->exp ring; psA = 1 bank, one HEAD
        # at a time (denominator rides column 32 of each 33-col AV group);
        # psD = 1 bank for the deferred v/gate/qk1 projections so they never
        # steal a psS slot.  Deferred work is chunked <=2 matmuls per stream
        # position so the in-order PE queue never delays a QK by more than
        # ~0.4us.  Drains split: normalize right after the head's last AV;
        # the pool gate-multiply waits until all gate groups exist.
        psA_cm = tc.tile_pool(name="psA", bufs=1, space="PSUM")
        psA = psA_cm.__enter__()
        psD_cm = tc.tile_pool(name="psD", bufs=1, space="PSUM")
        psD = psD_cm.__enter__()
        psS_cm = tc.tile_pool(name="psS", bufs=3, space="PSUM")
        psS = psS_cm.__enter__()

        psA_t = {}
        eT = {}

        def issue_av(h, kk):
            if h not in psA_t:
                psA_t[h] = psA.tile([P, 512], f32, tag="a", name=f"pa{h}")
            pa = psA_t[h]
            e = eT[(h, kk)]
            first = (kk == 0)
            last = (kk == NLT - 1)
            for qt in range(NLT):
                lhs = e[:, qt * P:(qt + 1) * P]
                nc.tensor.matmul(pa[:, qt * CA:(qt + 1) * CA], lhs,
                                 vaug[:, kk, h * CA:(h + 1) * CA],
                                 start=(first and qt == 0),
                                 stop=(last and qt == NLT - 1))

        def norm_head(h):
            """attn = pa / denom -> agall (ungated)."""
            pa = psA_t[h]
            pav = pa[:, 0:NLT * CA].rearrange("p (q c) -> p q c", c=CA)
            rec = work.tile([P, NLT], f32, tag="rec", name=f"rec{h}")
            nc.vector.reciprocal(rec[:], pav[:, :, C])
            cs = slice(h * C, (h + 1) * C)
            nc.vector.tensor_tensor(
                agall[:, :, cs], pav[:, :, 0:C],
                rec[:].unsqueeze(2).broadcast_to([P, NLT, C]), op=ALU.mult)

        def gate_head(h, eng):
            cs = slice(h * C, (h + 1) * C)
            eng.tensor_tensor(agall[:, :, cs], agall[:, :, cs],
                              gateall[:, :, cs], op=ALU.mult)

        def fine_tail(h):
            """head 7: per-qt normalize+gate+transpose+out."""
            pa = psA_t[h]
            pav = pa[:, 0:NLT * CA].rearrange("p (q c) -> p q c", c=CA)
            rec = work.tile([P, NLT], f32, tag="rec", name=f"rec{h}")
            nc.vector.reciprocal(rec[:], pav[:, :, C])
            cs = slice(h * C, (h + 1) * C)
            for qt in range(NLT):
                agv = agall[:, qt:qt + 1, cs]
                nc.vector.tensor_tensor(
                    agv, pav[:, qt:qt + 1, 0:C],
                    rec[:, qt:qt + 1].unsqueeze(2).broadcast_to([P, 1, C]),
                    op=ALU.mult)
                nc.vector.tensor_tensor(agv, agv, gateall[:, qt:qt + 1, cs],
                                        op=ALU.mult)
                tp = psS.tile([P, P], bf16, tag="s", name=f"tp{qt}")
                nc.tensor.transpose(tp[:], agall[:, qt, P:2 * P], identb[:])
                nc.vector.tensor_copy(agT[1][:, qt * P:(qt + 1) * P], tp[:])
                out_tile(qt)

        def out_tile(i):
            ps = psS.tile([P, 256], f32, tag="s", name=f"po{i}")
            nc.tensor.matmul(ps[:], onesf[:], bob[:, 0:256],
                             start=True, stop=False)
            for j in range(NHC):
                nc.tensor.matmul(ps[:], agT[j][:, i * P:(i + 1) * P],
                                 wo[:, j, :], start=False, stop=(j == 1))
            o = opool.tile([P, 256], f32, tag="o", name=f"ot{i}")
            # alternate the PSUM->SBUF copy between ACT and DVE in the tail
            if i % 2 == 0:
                nc.scalar.activation(o[:], ps[:], AF.Copy)
            else:
                nc.vector.tensor_copy(o[:], ps[:])
            eng = nc.gpsimd if i % 2 == 0 else nc.sync
            eng.dma_start(out_e.ap()[i * P:(i + 1) * P, :], o[:])

        # ---- deferred projection chunks (psD, <=2 MMs each) ----
        psD_t = {}

        def v_chunk(pr, half):
            # half 0: tile 2pr MMs; half 1: tile 2pr+1 MMs + strided copy
            if half == 0:
                psD_t["v"] = psD.tile([P, 2, HC], f32, tag="d", name=f"pv{pr}")
            ps = psD_t["v"]
            i = 2 * pr + half
            for jj in range(NFC):
                nc.tensor.matmul(ps[:, half, :],
                                 xT[jj][:, i * P:(i + 1) * P], wv[:, jj, :],
                                 start=(jj == 0), stop=(jj == 1))
            if half == 1:
                dst = (vaug[:, 2 * pr:2 * pr + 2, :]
                       .rearrange("p t (h c) -> p t h c", c=CA)[:, :, :, 0:C])
                src = ps[:].rearrange("p t (h c) -> p t h c", c=C)
                nc.scalar.activation(dst, src, AF.Copy)

        def g_chunk(pr, half):
            if half == 0:
                psD_t["g"] = psD.tile([P, 2, HC], f32, tag="d", name=f"pg{pr}")
            ps = psD_t["g"]
            i = 2 * pr + half
            for jj in range(NFC):
                nc.tensor.matmul(ps[:, half, :],
                                 xT[jj][:, i * P:(i + 1) * P], wg[:, jj, :],
                                 start=(jj == 0), stop=(jj == 1))
            if half == 1:
                dst = gateall[:, 2 * pr:2 * pr + 2, :]
                nc.scalar.activation(dst, ps[:], AF.Tanh, scale=0.5)
                nc.gpsimd.tensor_scalar(dst, dst, 1.0, None, op0=ALU.add)

        def qk1_chunk(qk, m):
            # one m-half of q/k chunk 1 (2 MMs) + its copy
            w_ = wq if qk == 0 else wk
            ps = psD.tile([P, 512], f32, tag="d", name=f"pqk1_{qk}{m}")
            ms = slice(512 * m, 512 * (m + 1))
            for jj in range(NFC):
                nc.tensor.matmul(ps[:], w_[:, jj, P:2 * P], xT[jj][:, ms],
                                 start=(jj == 0), stop=(jj == 1))
            nc.scalar.activation(qkT[1][:, qk, ms], ps[:], AF.Copy)

        extras = {
            (0, 0): lambda: v_chunk(0, 0), (0, 1): lambda: v_chunk(0, 1),
            (0, 2): lambda: v_chunk(1, 0), (0, 3): lambda: v_chunk(1, 1),
            (0, 4): lambda: v_chunk(2, 0), (0, 5): lambda: v_chunk(2, 1),
            (0, 6): lambda: v_chunk(3, 0), (0, 7): lambda: v_chunk(3, 1),
            (1, 0): lambda: g_chunk(0, 0), (1, 1): lambda: g_chunk(0, 1),
            (1, 2): lambda: g_chunk(1, 0), (1, 3): lambda: g_chunk(1, 1),
            (1, 4): lambda: g_chunk(2, 0), (1, 5): lambda: g_chunk(2, 1),
            (1, 6): lambda: g_chunk(3, 0), (1, 7): lambda: g_chunk(3, 1),
            (2, 5): lambda: qk1_chunk(1, 0), (2, 7): lambda: qk1_chunk(1, 1),
            (3, 1): lambda: qk1_chunk(0, 0), (3, 5): lambda: qk1_chunk(0, 1),
            # gate multiplies (pool) after norm_head(h) (pops at (h+1,2))
            (2, 1): lambda: gate_head(0, nc.gpsimd),
            (2, 3): lambda: gate_head(1, nc.gpsimd),
            (3, 3): lambda: gate_head(2, nc.gpsimd),
            (4, 3): lambda: gate_head(3, nc.gpsimd),
            (5, 3): lambda: gate_head(4, nc.gpsimd),
            (6, 3): lambda: gate_head(5, nc.gpsimd),
            (7, 3): lambda: gate_head(6, nc.gpsimd),
        }
        # agT[0] DMA transposes once heads 0..3 are gated
        def agt0(q0, q1):
            for qt in range(q0, q1):
                nc.sync.dma_start_transpose(agT[0][:, qt * P:(qt + 1) * P],
                                            agall[:, qt, 0:P])
        extras[(5, 2)] = lambda: agt0(0, 4)
        extras[(5, 4)] = lambda: agt0(4, 8)

        # AV issued with LAG 2 behind the exp stream: the PE queue is
        # in-order, so an AV waiting on exp(i-1) would block QK(i+1) whose
        # own dependency (slot of exp(i-2)) is already satisfied.
        pending = []

        def tick_av():
            t = pending.pop(0)
            issue_av(*t)
            if t[1] == NLT - 1 and t[0] < H - 1:
                norm_head(t[0])

        for h in range(H):
            jh, ph = h // 4, 32 * (h % 4)
            hp = slice(ph, ph + 32)
            for kk in range(NLT):
                sp = psS.tile([P, L], f32, tag="s", name=f"sp{h}_{kk}")
                for m in range(2):
                    ms = slice(512 * m, 512 * (m + 1))
                    nc.tensor.matmul(sp[:, ms],
                                     kTs[jh][hp, kk * P:(kk + 1) * P],
                                     qTs[jh][hp, ms], start=True, stop=True,
                                     tile_position=(ph, 0))
                e = epool.tile([P, L], bf16, tag="e", name=f"e{h}_{kk}")
                bcol = kk * H + h
                if act_on[h * NLT + kk]:
                    nc.scalar.activation(e[:], sp[:], AF.Exp,
                                         bias=bTsb[:, bcol:bcol + 1])
                else:
                    nc.vector.tensor_scalar(e[:].bitcast(i16), sp[:], A_SCH,
                                            sbT[:, bcol:bcol + 1],
                                            op0=ALU.mult, op1=ALU.add)
                eT[(h, kk)] = e
                pending.append((h, kk))
                if len(pending) > 3:
                    tick_av()
                if (h, kk) in extras:
                    extras[(h, kk)]()
        while pending:
            tick_av()
        fine_tail(H - 1)

        psS_cm.__exit__(None, None, None)
        psD_cm.__exit__(None, None, None)
        psA_cm.__exit__(None, None, None)

    # Pin Exp/Tanh/Copy to the one combined table set (single load).
    import concourse.bacc as bacc_mod
    orig_gat = bacc_mod.get_activation_tables

    def gat_combined(arch):
        t = orig_gat(arch)
        return {name: (funcs if name == "exp_and_others" else set())
                for name, funcs in t.items()}

    bacc_mod.get_activation_tables = gat_combined
    try:
        nc.compile()
    finally:
        bacc_mod.get_activation_tables = orig_gat
    return nc


def _prep_inputs(features, ln_g, ln_b, Wq, bq, Wk, bk, Wv, bv, Wb, bb,
                 Wg, bg, Wo, bo):
    import ml_dtypes
    bf = ml_dtypes.bfloat16
    f32 = np.float32
    sq = f32(1.0 / np.sqrt(C))
    g_ = np.asarray(ln_g, f32)[:, None]
    beta = np.asarray(ln_b, f32)

    Wq_ = np.asarray(Wq, f32) * g_ * sq
    Wk_ = np.asarray(Wk, f32) * g_
    Wv_ = np.asarray(Wv, f32) * g_
    Wg_ = np.asarray(Wg, f32) * g_
    bq_t = (beta @ np.asarray(Wq, f32) + np.asarray(bq, f32)) * sq  # [HC]
    bv_ = beta @ np.asarray(Wv, f32) + np.asarray(bv, f32)
    bg_ = beta @ np.asarray(Wg, f32) + np.asarray(bg, f32)
    assert np.abs(bv_).max() == 0.0, "nonzero v bias path not built"
    assert np.abs(bg_).max() == 0.0, "nonzero gate bias path not built"
    # per-key bias: Wb fold + q-bias cross term (softmax-invariant parts drop)
    WB = np.asarray(Wb, f32) * g_
    for h in range(H):
        WB[:, h] += Wk_[:, C * h:C * (h + 1)] @ bq_t[C * h:C * (h + 1)]
    BB = beta @ np.asarray(Wb, f32) + np.asarray(bb, f32)  # [H]

    def wsplit(W, n, dt_):
        return np.ascontiguousarray(
            np.asarray(W, f32).reshape(NFC, P, n).transpose(1, 0, 2)).astype(dt_)

    common = {
        "wq": wsplit(Wq_, HC, bf),
        "wk": wsplit(Wk_, HC, bf),
        "wv": wsplit(Wv_, HC, bf),
        "wg": wsplit(Wg_, HC, bf),
        "wb": wsplit(WB, H, bf),
        "wo": wsplit(np.asarray(Wo, f32) * 0.5, F, bf),
        "bbb": np.ascontiguousarray(np.tile(BB, (P, 1))).astype(f32),
        "idb": np.eye(P, dtype=np.float32).astype(bf),
        "bob": np.ascontiguousarray(
            np.tile(np.asarray(bo, f32), (1, 2))).astype(bf),
    }
    feats = np.asarray(features, f32)
    in_maps = []
    for b_ in range(N_CORES):
        m = dict(common)
        m["feat"] = np.ascontiguousarray(feats[:, b_, :])
        in_maps.append(m)
    return in_maps


def kernel(**inputs):
    from concourse.bass_utils import run_bass_kernel_spmd

    if "nc" not in _COMPILED:
        _COMPILED["nc"] = _build()
    nc = _COMPILED["nc"]
    in_maps = _prep_inputs(**inputs)
    res = run_bass_kernel_spmd(nc, in_maps, list(range(N_CORES)))
    out = np.stack([res.results[b_]["out"] for b_ in range(N_CORES)], axis=1)
    return np.ascontiguousarray(out.astype(np.float32))
